# revision 13
# baseline (speedup 1.0000x reference)
"""MoE expert MLP (SwiGLU, top-2 routing) on 8 Trainium2 NeuronCores.

Strategy: expert-parallel. Host routes tokens (stable argsort by expert id,
matching the reference), gathers each expert's token rows, and pads them to a
fixed capacity C. Core e runs expert e's two GEMMs + SwiGLU over its C-column
token panel; the host scatters results back into the permuted [N, H] output.

Design (trace-driven; exec_time = last-epilogue-inst-end minus first-main-
block-inst; the ~6us framework preamble is excluded but the fixed ~8us
256-semaphore wipe epilogue IS counted, so only the work window matters):
  - All per-core input bytes stream in exact PE consumption order on the
    sync HWDGE ring: [xT+a0 (raw) | x-tail+a0-tail+b0 (raw) | w1 pairs |
    a-tail + fp8 b-tail | w2 fp8]. Per-chunk column-slice DMAs into
    persistent tiles give Tile-tracked gating with no buffer rotation.
  - PE warmup: ~40 garbage N=128 matmuls issued before the input-gate wait
    run during the 6.9-10.9us framework-preamble/DMA-ramp window, tripping
    the HAM activity monitor so every real matmul runs at 2.4GHz (without
    this the PE runs at K=4/8 = 1.2GHz until ~18us: +~3us).
  - The warmup uses N=512 so it is only 18 Tensor-stream instructions:
    queue 14 (engine instruction fetch, 16KB chunks) rides exclusively on
    the first SDMA engine of the core's block and steals from its input
    bandwidth mid-stream, so program size is kept small.
  - Quantization (e3m4 = TRN FP8_EXP3, 4 mantissa bits): all of w2 and the
    b-halves of pairs 4-15 (NFP8B=12) are stored *64 in fp8, cutting 3.7MB
    (~28%) off the 9.2MB stream. The 1/64 compensation folds into the bf16
    b-halves (exponent shift, lossless) and a 2^-12 scalar on sa for the
    fp8-b pairs. Measured rel err 0.018656 vs the 0.02 gate (NFP8B=14 ->
    0.0193 leaves <4% margin - declined).
  - C=142 = the exact max expert load for this routing (ceil to mult of 2).
  - GEMM2 stores per h-block, alternating the scalar/sync HWDGE rings so
    the final store's trigger never queues behind earlier store triggers;
    the ~1.3us HBM write-receipt of the last store is the irreducible tail.

Per-core dataflow (all activations transposed, tokens on the free dim):
  GEMM1:  h1T[m-block j] = w1[e][:, cols].T @ xT      (8 H k-tiles, PSUM acc)
  SwiGLU: interT[j] = silu(h1T_a[j]) * h1T_b[j]       (ACT + DVE out of PSUM)
  GEMM2:  yT[h-block]  = w2q[e][:, cols].T @ interT   (16 I k-tiles, fp8 lhsT)
"""

import numpy as np
import ml_dtypes

import concourse.bass as bass
import concourse.mybir as mybir
import concourse.tile as tile
from concourse import bacc
from concourse.bass_utils import run_bass_kernel_spmd

BF16 = mybir.dt.bfloat16
FP8 = mybir.dt.float8e3
F32 = mybir.dt.float32
NP_BF16 = ml_dtypes.bfloat16
NP_FP8 = ml_dtypes.float8_e3m4

# Problem shape (hardcoded per the contract; matches nn_Experts_41429254537622)
B, S, H, I, E, TOPK = 1, 512, 1024, 2048, 8, 2
N_CORES = 8
KH = H // 128    # 8  k-tiles for GEMM1 (contraction over H)
NPAIR = I // 128 # 16 (a, b) pairs of 128-wide w1 column blocks
KI = I // 128    # 16 k-tiles for GEMM2 (contraction over I)
MH = H // 128    # 8  output row blocks of yT
PAIR_COLS = 2 * KH * 128   # 2048 w1 blob columns per (a, b) pair block
HB_COLS = KI * 128         # 2048 w2 blob columns per h block (fp8: 1B each)
W2_SCALE = 64.0            # w2 stored as e3m4 * 64; 1/64 folded into w1's b half
NFP8B = 12                 # trailing pairs whose b half is stored fp8 (b*64)
BQ0 = NPAIR - NFP8B        # first fp8-b pair
BQ_COLS = KH * 128         # 1024 fp8 b columns per pair
NWARM = 9                  # PE warmup matmuls, N=512 (~3.8us at cold 1.2GHz)

_compiled = {}
LAST_RUNS = []  # BassKernelResults of the most recent kernel() call (for test harness)


def _build_program(C):
    XCOLS = KH * C
    nc = bacc.Bacc(
        "TRN2", target_bir_lowering=False, debug=False, num_devices=N_CORES
    )
    # blob1: [ xT (XCOLS) | pair0 a+b (PAIR_COLS) | pairs 1..15 ] in exact
    # consumption order. blob2: w2 as fp8, h-block-major.
    blob1 = nc.dram_tensor(
        "blob1",
        [128, XCOLS + BQ0 * PAIR_COLS + NFP8B * BQ_COLS],
        BF16,
        kind="ExternalInput",
    )
    blob2 = nc.dram_tensor(
        "blob2", [128, NFP8B * BQ_COLS + MH * HB_COLS], FP8,
        kind="ExternalInput",
    )
    yT_d = nc.dram_tensor("yT", [128, MH * C], BF16, kind="ExternalOutput")

    # Pre-Tile raw load: the PE's first work (pair 0's a-chain) needs only
    # x k0-3 + a0 k0-3, so that prefix is its own DMA streamed during the
    # framework preamble. The rest of the head rides as the first Tile
    # chunks (Tile gates the a-chain mid-accumulation, which is legal).
    # blob1 head: [x k0-3 (4C) | a0 k0-3 (512) | x k4-7 (4C) | a0 k4-7
    # (512) | b0 (1024)]
    HC = XCOLS // 2                 # 4C
    P0A = HC + 4 * 128              # end of [x k0-3 | a0 k0-3]
    P0B = XCOLS + PAIR_COLS         # end of the whole head
    xw0_raw = nc.alloc_sbuf_tensor("xw0_pre", [128, P0A], BF16)
    warm_sb = nc.alloc_sbuf_tensor("warm_sb", [128, 512 + 128], BF16)
    pre_sem = nc.alloc_semaphore(name="pre_dma_sem")
    xw0 = xw0_raw.ap()
    nc.sync.dma_start(xw0[:, :P0A], blob1[:, :P0A]).then_inc(pre_sem, 16)

    # PE warmup: wide (N=512) garbage matmuls into an aliased PSUM bank.
    # They run during the preamble + head-DMA window (strictly before every
    # real matmul in PE program order) and hold the PE busy >3.4us so HAM
    # unthrottles the clock to 2.4GHz before real work starts. Bank 7 is
    # reused by a Tile PSUM pool much later; PE-queue ordering makes the
    # aliasing safe. N=512 keeps this at 9 LDWEIGHTS+MATMUL pairs so the
    # Tensor instruction stream (fetched over the straggler-prone first
    # SDMA engine) stays small.
    warm_ps = nc.place_psum_tensor("warm_ps", [128, 512], F32, bank=7)
    wps = warm_ps.ap()
    wsb = warm_sb.ap()
    for _ in range(NWARM):
        nc.tensor.matmul(
            wps[:, :], wsb[:, 512:640], wsb[:, :512], start=True, stop=True
        )
    nc.tensor.wait_ge(pre_sem, 16)

    with tile.TileContext(nc) as tc:
        with (
            tc.tile_pool(name="wp", bufs=1) as wp,
            tc.tile_pool(name="sap", bufs=4) as sap,
            tc.tile_pool(name="outp", bufs=3) as outp,
            tc.tile_pool(name="ps1", bufs=6, space="PSUM") as ps1,
            tc.tile_pool(name="ps2", bufs=2, space="PSUM") as ps2,
        ):
            # Persistent weight panels + inter panel: DMAs write disjoint
            # column slices, matmuls read sub-slices; Tile's range tracker
            # gives per-chunk gating with no buffer rotation or WAR stalls.
            # w1t holds [pairs 1..3 full | a4..a15]; b4..b15 live fp8 in
            # w1bq (stored *64, compensated by 2^-12 on sa).
            # xb1 carries [x k4-7 | a0 k4-7 | b0] as the first Tile chunks.
            # w2t is split-contraction: [half1 = per-h k-tiles 0-7 | half2 =
            # per-h k-tiles 8-15]; yh holds the f32 half1 partial sums.
            xb1 = wp.tile([128, HC + 4 * 128 + KH * 128], BF16)
            w1t = wp.tile(
                [128, (BQ0 - 1) * PAIR_COLS + NFP8B * BQ_COLS], BF16
            )
            w1bq = wp.tile([128, NFP8B * BQ_COLS], FP8)
            w2t = wp.tile([128, MH * HB_COLS], FP8)
            it_all = wp.tile([128, KI * C], BF16)
            yh = wp.tile([128, MH * C], F32)
            A8 = (BQ0 - 1) * PAIR_COLS          # w1t col of the a-tail
            HB2 = KI // 2 * 128                 # 1024 w2 cols per (h, half)
            H2 = MH * HB2                       # w2t col base of half2

            def xk(k):
                # x k-tile k: first half in the pre tensor, rest in xb1.
                if k < 4:
                    return xw0[:, k * C:(k + 1) * C]
                return xb1[:, (k - 4) * C:(k - 3) * C]

            # Head tail as two Tile chunks (split so pair 0's k4 gate lands
            # before b0), then w1 pairs 1..3, then per 2 fp8-b pairs an
            # a-chunk (bf16) + b-chunk (fp8), then w2 per h-block; all
            # back-to-back on the sync ring in consumption order.
            XB_SPLIT = HC + 4 * 128
            nc.sync.dma_start(xb1[:, :XB_SPLIT], blob1[:, P0A:P0A + XB_SPLIT])
            nc.sync.dma_start(
                xb1[:, XB_SPLIT:], blob1[:, P0A + XB_SPLIT:P0B]
            )
            W1BASE = P0B                        # blob1 col of pair 1
            for p0 in range(1, BQ0):
                nc.sync.dma_start(
                    w1t[:, (p0 - 1) * PAIR_COLS:p0 * PAIR_COLS],
                    blob1[:, W1BASE + (p0 - 1) * PAIR_COLS:W1BASE + p0 * PAIR_COLS],
                )
            AB = W1BASE + (BQ0 - 1) * PAIR_COLS  # blob1 col of the a-tail
            W2B = NFP8B * BQ_COLS               # blob2 col of w2
            for jj in range(0, NFP8B, 2):
                nc.sync.dma_start(
                    w1t[:, A8 + jj * BQ_COLS:A8 + (jj + 2) * BQ_COLS],
                    blob1[:, AB + jj * BQ_COLS:AB + (jj + 2) * BQ_COLS],
                )
                nc.sync.dma_start(
                    w1bq[:, jj * BQ_COLS:(jj + 2) * BQ_COLS],
                    blob2[:, jj * BQ_COLS:(jj + 2) * BQ_COLS],
                )
                if jj >= 4:
                    # w2 half1 for h-blocks (jj-4, jj-3): consumed by the
                    # split-GEMM2 slices interleaved after pairs 4+jj, 5+jj.
                    hb = (jj - 4) * HB2
                    nc.sync.dma_start(
                        w2t[:, hb:hb + 2 * HB2],
                        blob2[:, W2B + hb:W2B + hb + 2 * HB2],
                    )
            for h in range(MH):
                nc.sync.dma_start(
                    w2t[:, H2 + h * HB2:H2 + (h + 1) * HB2],
                    blob2[:, W2B + H2 + h * HB2:W2B + H2 + (h + 1) * HB2],
                )

            # GEMM1 + SwiGLU, pair-by-pair in stream order.
            for j in range(NPAIR):
                if j == 0:
                    asrc, abase = None, 0            # per-k split below
                    bsrc, bbase = xb1, HC - 4 * 128  # b0 at xb1[HC+512:]
                elif j < BQ0:
                    asrc, abase = w1t, (j - 1) * PAIR_COLS
                    bsrc, bbase = w1t, (j - 1) * PAIR_COLS
                else:
                    asrc, abase = w1t, A8 + (j - BQ0) * BQ_COLS
                    bsrc, bbase = w1bq, (j - BQ0) * BQ_COLS - KH * 128
                pa = ps1.tile([128, C], F32, tag="pab")
                pb = ps1.tile([128, C], F32, tag="pab")
                for k in range(KH):
                    if j == 0:
                        if k < 4:
                            ak = xw0[:, HC + k * 128:HC + (k + 1) * 128]
                        else:
                            ak = xb1[:, HC + (k - 4) * 128:HC + (k - 3) * 128]
                    else:
                        ak = asrc[:, abase + k * 128:abase + (k + 1) * 128]
                    nc.tensor.matmul(
                        pa[:],
                        ak,
                        xk(k),
                        start=(k == 0),
                        stop=(k == KH - 1),
                    )
                for k in range(KH):
                    nc.tensor.matmul(
                        pb[:],
                        bsrc[:, bbase + (KH + k) * 128:bbase + (KH + k + 1) * 128],
                        xk(k),
                        start=(k == 0),
                        stop=(k == KH - 1),
                    )
                sa = sap.tile([128, C], F32, tag="sa")
                nc.scalar.activation(
                    sa[:], pa[:], mybir.ActivationFunctionType.Silu
                )
                if j >= BQ0:
                    # b was stored *64 instead of /64: fold 2^-12 into sa.
                    nc.vector.tensor_scalar_mul(sa[:], sa[:], 2.0 ** -12)
                nc.vector.tensor_mul(it_all[:, j * C:(j + 1) * C], sa[:], pb[:])

                # Split-contraction GEMM2, half 1 (k-tiles 0-7, i.e. pairs
                # 0-7 of it_all, all ready once pair 7 retired): one h-block
                # slice after each of pairs 8-15. This fills the PE stalls
                # the bf16/fp8 weight stream cannot avoid (pair demand is
                # ~1.4x the stream rate) and keeps HAM at full clock; the
                # f32 partials park in yh.
                if j >= NPAIR - MH:
                    h = j - (NPAIR - MH)
                    py = ps2.tile([128, C], F32, tag="py")
                    for ki in range(KI // 2):
                        nc.tensor.matmul(
                            py[:],
                            w2t[:, h * HB2 + ki * 128:h * HB2 + (ki + 1) * 128],
                            it_all[:, ki * C:(ki + 1) * C],
                            start=(ki == 0),
                            stop=(ki == KI // 2 - 1),
                        )
                    nc.vector.tensor_copy(yh[:, h * C:(h + 1) * C], py[:])

            # GEMM2 half 2 (k-tiles 8-15): consumes the stream tail at the
            # rate it arrives; DVE adds the half1 partials and casts to
            # bf16. Per-h-block stores alternate the scalar/sync HWDGE
            # rings so the last store's trigger issues immediately after
            # its add.
            for h in range(MH):
                py = ps2.tile([128, C], F32, tag="py")
                for ki in range(KI // 2):
                    nc.tensor.matmul(
                        py[:],
                        w2t[:, H2 + h * HB2 + ki * 128:H2 + h * HB2 + (ki + 1) * 128],
                        it_all[:, (KI // 2 + ki) * C:(KI // 2 + ki + 1) * C],
                        start=(ki == 0),
                        stop=(ki == KI // 2 - 1),
                    )
                yt = outp.tile([128, C], BF16, tag="yt")
                nc.vector.tensor_add(yt[:], yh[:, h * C:(h + 1) * C], py[:])
                eng = nc.scalar if h % 2 == 0 else nc.sync
                eng.dma_start(yT_d[:, h * C:(h + 1) * C], yt[:])
    nc.compile()
    return nc


def _get_program(C):
    if C not in _compiled:
        _compiled[C] = _build_program(C)
    return _compiled[C]


def _relayout_w1(w1_e):
    # w1_e: [H, 2I] bf16 (b-half of pairs < BQ0 pre-scaled by 1/64) ->
    # [128, BQ0*PAIR_COLS + NFP8B*BQ_COLS]: pairs 0..BQ0-1 hold a_j's 8
    # k-tiles then b_j's; pairs BQ0.. hold only a_j (their b is fp8 in
    # blob2). Stationary [K=128, M=128] layout (partition = contraction row).
    A = w1_e[:, :I].reshape(KH, 128, NPAIR, 128)
    Bh = w1_e[:, I:].reshape(KH, 128, NPAIR, 128)
    pairs = np.stack([A[:, :, :BQ0], Bh[:, :, :BQ0]], axis=0)
    full = pairs.transpose(2, 3, 0, 1, 4).reshape(128, BQ0 * PAIR_COLS)
    atail = A[:, :, BQ0:].transpose(1, 2, 0, 3).reshape(128, NFP8B * BQ_COLS)
    return np.ascontiguousarray(np.concatenate([full, atail], axis=1))


def _relayout_w1bq(bq_e):
    # bq_e: [H, NFP8B*128] fp8 (*64) -> [128, NFP8B*BQ_COLS], per-pair the
    # 8 k-tiles in stationary layout.
    r = bq_e.reshape(KH, 128, NFP8B, 128)
    return np.ascontiguousarray(
        r.transpose(1, 2, 0, 3).reshape(128, NFP8B * BQ_COLS)
    )


def _relayout_w2(w2_e):
    # w2_e: [I, H] fp8 -> [128, MH*HB_COLS] in split-contraction order:
    # [half1 = h-major k-tiles 0-7 | half2 = h-major k-tiles 8-15], each
    # h slice holding its 8 stationary k-tiles in consumption order.
    r = w2_e.reshape(KI, 128, MH, 128)
    h1 = r[:KI // 2].transpose(1, 2, 0, 3).reshape(128, MH * KI // 2 * 128)
    h2 = r[KI // 2:].transpose(1, 2, 0, 3).reshape(128, MH * KI // 2 * 128)
    return np.ascontiguousarray(np.concatenate([h1, h2], axis=1))


def kernel(hidden_states, tokens_per_expert, w1, w2):
    x = np.asarray(hidden_states).reshape(-1, H)
    flat = np.asarray(tokens_per_expert).reshape(-1).astype(np.int64)
    w1 = np.asarray(w1)
    w2 = np.asarray(w2)
    n_rows = flat.shape[0]

    order = np.argsort(flat, kind="stable")
    token_of_row = order // TOPK
    counts = np.bincount(flat, minlength=E)
    starts = np.concatenate([[0], np.cumsum(counts)[:-1]])

    x_bf = x.astype(NP_BF16)
    if w1.dtype != NP_BF16:
        w1 = w1.astype(NP_BF16)

    C = max(48, int(-(-int(counts.max()) // 2)) * 2)
    XCOLS = KH * C
    nc = _get_program(C)

    # b-half of bf16 pairs scaled by 1/W2_SCALE (exponent shift, lossless);
    # w2 and the trailing b-halves stored as e3m4 * W2_SCALE (the latter
    # compensated by 2^-12 on sa in-kernel).
    w1s = np.concatenate(
        [w1[:, :, :I], (w1[:, :, I:].astype(np.float32) / W2_SCALE).astype(NP_BF16)],
        axis=2,
    )
    bq = (
        w1[:, :, I + BQ0 * 128:].astype(np.float32) * W2_SCALE
    ).astype(NP_FP8)
    w2q = (w2.astype(np.float32) * W2_SCALE).astype(NP_FP8)
    w1r = [_relayout_w1(w1s[e]) for e in range(E)]
    bqr = [_relayout_w1bq(bq[e]) for e in range(E)]
    w2r = [_relayout_w2(w2q[e]) for e in range(E)]

    out = np.zeros((n_rows, H), dtype=NP_BF16)
    LAST_RUNS.clear()
    n_waves = int(max(1, -(-int(counts.max()) // C)))
    for wave in range(n_waves):
        in_maps = []
        for e in range(E):
            lo = starts[e] + wave * C
            cnt = int(min(C, max(0, counts[e] - wave * C)))
            xe = np.zeros((C, H), dtype=NP_BF16)
            if cnt:
                xe[:cnt] = x_bf[token_of_row[lo:lo + cnt]]
            # xT layout: [128, KH*C], k-tile k at cols [k*C, (k+1)*C):
            # xT[p, k*C + c] = xe[c, k*128 + p]
            xT = np.ascontiguousarray(
                xe.T.reshape(KH, 128, C).transpose(1, 0, 2).reshape(128, XCOLS)
            )
            blob1 = np.concatenate(
                [
                    xT[:, :XCOLS // 2],        # x k0-3
                    w1r[e][:, :512],           # a0 k0-3
                    xT[:, XCOLS // 2:],        # x k4-7
                    w1r[e][:, 512:1024],       # a0 k4-7
                    w1r[e][:, 1024:2048],      # b0
                    w1r[e][:, 2048:],          # pairs 1.. + a-tail
                ],
                axis=1,
            )
            blob2 = np.concatenate([bqr[e], w2r[e]], axis=1)
            in_maps.append({"blob1": blob1, "blob2": blob2})

        res = run_bass_kernel_spmd(nc, in_maps, list(range(N_CORES)))
        LAST_RUNS.append(res)
        for e in range(E):
            lo = starts[e] + wave * C
            cnt = int(min(C, max(0, counts[e] - wave * C)))
            if not cnt:
                continue
            yT = res.results[e]["yT"]
            # yT[p, h*C + c] = y[c, h*128 + p]
            y = yT.reshape(128, MH, C).transpose(2, 1, 0).reshape(C, H)
            out[lo:lo + cnt] = y[:cnt]
    return out


# revision 17
# speedup vs baseline: 1.0294x; 1.0294x over previous
"""MoE expert MLP (SwiGLU, top-2 routing) on 8 Trainium2 NeuronCores.

Strategy: expert-parallel. Host routes tokens (stable argsort by expert id,
matching the reference), gathers each expert's token rows, and pads them to a
fixed capacity C. Core e runs expert e's two GEMMs + SwiGLU over its C-column
token panel; the host scatters results back into the permuted [N, H] output.

Design (trace-driven; exec_time = last-epilogue-inst-end minus first-main-
block-inst; the ~6us framework preamble is excluded but the fixed ~8.3us
256-semaphore wipe epilogue IS counted, so only the work window matters):
  - All per-core input bytes stream in exact PE consumption order on the
    sync HWDGE ring: [xT+a0 (raw pre-chunk) | x-tail+a0-tail+b0 | w1
    pairs | a-tail + fp8 b-tail interleaved with w2-half1 | w2-half2].
    Per-chunk column-slice DMAs into persistent tiles give Tile-tracked
    gating; chunk count and store structure match the long-proven session
    baseline exactly (26 input DMAs + 4 stores) — a variant with 8
    per-h-block stores alternating sync/scalar rings and 12 w2 chunks hit
    nondeterministic NRT_EXEC_UNIT_UNRECOVERABLE hangs (2 of 9 runs).
  - PE warmup: 9 garbage N=512 matmuls issued before the input-gate wait
    run during the 6.9-10.6us framework-preamble/DMA-ramp window, tripping
    the HAM activity monitor so real matmuls run at 2.4GHz from the start
    (without this the PE runs at K=4/8 = 1.2GHz until ~18us: +~3us). Only
    18 Tensor-stream instructions: queue 14 (engine instruction fetch,
    16KB chunks) rides exclusively on the first SDMA engine of the core's
    block and steals from its input bandwidth mid-stream, so program size
    is kept small.
  - Split-contraction GEMM2: half 1 (it_all k-tiles 0-7, ready after pair
    7) runs as one h-block slice interleaved after each of pairs 8-15,
    parking f32 partials in SBUF; half 2 + DVE add + bf16 cast run after
    the pair loop. During the pair loop the weight stream cannot feed the
    warm PE (pair demand ~467GB/s bf16 / ~364GB/s fp8 vs ~330GB/s
    delivered), so without the interleave the PE idles in 1-3us slices
    (risking HAM re-throttle) and all 8.4us of GEMM2 serializes after the
    stream; with it the stalls are filled and only ~4.2us of half-2 GEMM2
    (consuming the tail at its ~250GB/s demand rate) remains at the end.
  - Quantization (e3m4 = TRN FP8_EXP3, 4 mantissa bits): all of w2 and the
    b-halves of pairs 4-15 (NFP8B=12) are stored *64 in fp8, cutting 3.7MB
    (~28%) off the 9.2MB stream. The 1/64 compensation folds into the bf16
    b-halves (exponent shift, lossless) and a 2^-12 scalar on sa for the
    fp8-b pairs. Measured rel err 0.018656 vs the 0.02 gate (NFP8B=14 ->
    0.0193 leaves <4% margin - declined).
  - C=142 = the exact max expert load for this routing (ceil to mult of 2).
  - GEMM2 stores per h-block, alternating the scalar/sync HWDGE rings so
    the final store's trigger never queues behind earlier store triggers;
    the ~1.3us HBM write-receipt of the last store is the irreducible tail.

Per-core dataflow (all activations transposed, tokens on the free dim):
  GEMM1:  h1T[m-block j] = w1[e][:, cols].T @ xT      (8 H k-tiles, PSUM acc)
  SwiGLU: interT[j] = silu(h1T_a[j]) * h1T_b[j]       (ACT + DVE out of PSUM)
  GEMM2:  yT[h-block]  = w2q[e][:, cols].T @ interT   (16 I k-tiles, fp8 lhsT)
"""

import numpy as np
import ml_dtypes

import concourse.bass as bass
import concourse.mybir as mybir
import concourse.tile as tile
from concourse import bacc
from concourse.bass_utils import run_bass_kernel_spmd

BF16 = mybir.dt.bfloat16
FP8 = mybir.dt.float8e3
F32 = mybir.dt.float32
NP_BF16 = ml_dtypes.bfloat16
NP_FP8 = ml_dtypes.float8_e3m4

# Problem shape (hardcoded per the contract; matches nn_Experts_41429254537622)
B, S, H, I, E, TOPK = 1, 512, 1024, 2048, 8, 2
N_CORES = 8
KH = H // 128    # 8  k-tiles for GEMM1 (contraction over H)
NPAIR = I // 128 # 16 (a, b) pairs of 128-wide w1 column blocks
KI = I // 128    # 16 k-tiles for GEMM2 (contraction over I)
MH = H // 128    # 8  output row blocks of yT
PAIR_COLS = 2 * KH * 128   # 2048 w1 blob columns per (a, b) pair block
HB_COLS = KI * 128         # 2048 w2 blob columns per h block (fp8: 1B each)
W2_SCALE = 64.0            # w2 stored as e3m4 * 64; 1/64 folded into w1's b half
NFP8B = 12                 # trailing pairs whose b half is stored fp8 (b*64)
BQ0 = NPAIR - NFP8B        # first fp8-b pair
BQ_COLS = KH * 128         # 1024 fp8 b columns per pair
NWARM = 9                  # PE warmup matmuls, N=512 (~3.8us at cold 1.2GHz)

_compiled = {}
LAST_RUNS = []  # BassKernelResults of the most recent kernel() call (for test harness)


def _build_program(C):
    XCOLS = KH * C
    nc = bacc.Bacc(
        "TRN2", target_bir_lowering=False, debug=False, num_devices=N_CORES
    )
    # blob1: [ xT (XCOLS) | pair0 a+b (PAIR_COLS) | pairs 1..15 ] in exact
    # consumption order. blob2: w2 as fp8, h-block-major.
    blob1 = nc.dram_tensor(
        "blob1",
        [128, XCOLS + BQ0 * PAIR_COLS + NFP8B * BQ_COLS],
        BF16,
        kind="ExternalInput",
    )
    blob2 = nc.dram_tensor(
        "blob2", [128, NFP8B * BQ_COLS + MH * HB_COLS], FP8,
        kind="ExternalInput",
    )
    yT_d = nc.dram_tensor("yT", [128, MH * C], BF16, kind="ExternalOutput")

    # Pre-Tile raw load: the PE's first work (pair 0's a-chain) needs only
    # x k0-3 + a0 k0-3, so that prefix is its own DMA streamed during the
    # framework preamble. The rest of the head rides as the first Tile
    # chunks (Tile gates the a-chain mid-accumulation, which is legal).
    # blob1 head: [x k0-3 (4C) | a0 k0-3 (512) | x k4-7 (4C) | a0 k4-7
    # (512) | b0 (1024)]
    HC = XCOLS // 2                 # 4C
    P0A = HC + 4 * 128              # end of [x k0-3 | a0 k0-3]
    P0B = XCOLS + PAIR_COLS         # end of the whole head
    xw0_raw = nc.alloc_sbuf_tensor("xw0_pre", [128, P0A], BF16)
    warm_sb = nc.alloc_sbuf_tensor("warm_sb", [128, 512 + 128], BF16)
    pre_sem = nc.alloc_semaphore(name="pre_dma_sem")
    xw0 = xw0_raw.ap()
    nc.sync.dma_start(xw0[:, :P0A], blob1[:, :P0A]).then_inc(pre_sem, 16)

    # PE warmup: wide (N=512) garbage matmuls into an aliased PSUM bank.
    # They run during the preamble + head-DMA window (strictly before every
    # real matmul in PE program order) and hold the PE busy >3.4us so HAM
    # unthrottles the clock to 2.4GHz before real work starts. Bank 7 is
    # reused by a Tile PSUM pool much later; PE-queue ordering makes the
    # aliasing safe. N=512 keeps this at 9 LDWEIGHTS+MATMUL pairs so the
    # Tensor instruction stream (fetched over the straggler-prone first
    # SDMA engine) stays small.
    warm_ps = nc.place_psum_tensor("warm_ps", [128, 512], F32, bank=7)
    wps = warm_ps.ap()
    wsb = warm_sb.ap()
    for _ in range(NWARM):
        nc.tensor.matmul(
            wps[:, :], wsb[:, 512:640], wsb[:, :512], start=True, stop=True
        )
    nc.tensor.wait_ge(pre_sem, 16)

    with tile.TileContext(nc) as tc:
        with (
            tc.tile_pool(name="wp", bufs=1) as wp,
            tc.tile_pool(name="sap", bufs=4) as sap,
            tc.tile_pool(name="outp", bufs=2) as outp,
            tc.tile_pool(name="ps1", bufs=6, space="PSUM") as ps1,
            tc.tile_pool(name="ps2", bufs=2, space="PSUM") as ps2,
        ):
            # Persistent weight panels + inter panel: DMAs write disjoint
            # column slices, matmuls read sub-slices; Tile's range tracker
            # gives per-chunk gating with no buffer rotation or WAR stalls.
            # w1t holds [pairs 1..3 full | a4..a15]; b4..b15 live fp8 in
            # w1bq (stored *64, compensated by 2^-12 on sa).
            # xb1 carries [x k4-7 | a0 k4-7 | b0] as the first Tile chunks.
            # w2t is split-contraction: [half1 = per-h k-tiles 0-7 | half2 =
            # per-h k-tiles 8-15]; yh holds the f32 half1 partial sums.
            xb1 = wp.tile([128, HC + 4 * 128 + KH * 128], BF16)
            w1t = wp.tile(
                [128, (BQ0 - 1) * PAIR_COLS + NFP8B * BQ_COLS], BF16
            )
            w1bq = wp.tile([128, NFP8B * BQ_COLS], FP8)
            w2t = wp.tile([128, MH * HB_COLS], FP8)
            it_all = wp.tile([128, KI * C], BF16)
            yh = wp.tile([128, MH * C], F32)
            A8 = (BQ0 - 1) * PAIR_COLS          # w1t col of the a-tail
            HB2 = KI // 2 * 128                 # 1024 w2 cols per (h, half)
            H2 = MH * HB2                       # w2t col base of half2

            def xk(k):
                # x k-tile k: first half in the pre tensor, rest in xb1.
                if k < 4:
                    return xw0[:, k * C:(k + 1) * C]
                return xb1[:, (k - 4) * C:(k - 3) * C]

            # Head tail as two Tile chunks (split so pair 0's k4 gate lands
            # before b0), then w1 pairs 1..3, then per 2 fp8-b pairs an
            # a-chunk (bf16) + b-chunk (fp8), then w2 per h-block; all
            # back-to-back on the sync ring in consumption order.
            XB_SPLIT = HC + 4 * 128
            nc.sync.dma_start(xb1[:, :XB_SPLIT], blob1[:, P0A:P0A + XB_SPLIT])
            nc.sync.dma_start(
                xb1[:, XB_SPLIT:], blob1[:, P0A + XB_SPLIT:P0B]
            )
            W1BASE = P0B                        # blob1 col of pair 1
            for p0 in range(1, BQ0):
                nc.sync.dma_start(
                    w1t[:, (p0 - 1) * PAIR_COLS:p0 * PAIR_COLS],
                    blob1[:, W1BASE + (p0 - 1) * PAIR_COLS:W1BASE + p0 * PAIR_COLS],
                )
            AB = W1BASE + (BQ0 - 1) * PAIR_COLS  # blob1 col of the a-tail
            W2B = NFP8B * BQ_COLS               # blob2 col of w2
            for jj in range(0, NFP8B, 2):
                nc.sync.dma_start(
                    w1t[:, A8 + jj * BQ_COLS:A8 + (jj + 2) * BQ_COLS],
                    blob1[:, AB + jj * BQ_COLS:AB + (jj + 2) * BQ_COLS],
                )
                nc.sync.dma_start(
                    w1bq[:, jj * BQ_COLS:(jj + 2) * BQ_COLS],
                    blob2[:, jj * BQ_COLS:(jj + 2) * BQ_COLS],
                )
                if jj >= 4:
                    # w2 half1 for h-blocks (jj-4, jj-3): consumed by the
                    # split-GEMM2 slices interleaved after pairs 4+jj, 5+jj.
                    hb = (jj - 4) * HB2
                    nc.sync.dma_start(
                        w2t[:, hb:hb + 2 * HB2],
                        blob2[:, W2B + hb:W2B + hb + 2 * HB2],
                    )
            for h in range(0, MH, 2):
                nc.sync.dma_start(
                    w2t[:, H2 + h * HB2:H2 + (h + 2) * HB2],
                    blob2[:, W2B + H2 + h * HB2:W2B + H2 + (h + 2) * HB2],
                )

            # GEMM1 + SwiGLU, pair-by-pair in stream order.
            for j in range(NPAIR):
                if j == 0:
                    asrc, abase = None, 0            # per-k split below
                    bsrc, bbase = xb1, HC - 4 * 128  # b0 at xb1[HC+512:]
                elif j < BQ0:
                    asrc, abase = w1t, (j - 1) * PAIR_COLS
                    bsrc, bbase = w1t, (j - 1) * PAIR_COLS
                else:
                    asrc, abase = w1t, A8 + (j - BQ0) * BQ_COLS
                    bsrc, bbase = w1bq, (j - BQ0) * BQ_COLS - KH * 128
                pa = ps1.tile([128, C], F32, tag="pab")
                pb = ps1.tile([128, C], F32, tag="pab")
                for k in range(KH):
                    if j == 0:
                        if k < 4:
                            ak = xw0[:, HC + k * 128:HC + (k + 1) * 128]
                        else:
                            ak = xb1[:, HC + (k - 4) * 128:HC + (k - 3) * 128]
                    else:
                        ak = asrc[:, abase + k * 128:abase + (k + 1) * 128]
                    nc.tensor.matmul(
                        pa[:],
                        ak,
                        xk(k),
                        start=(k == 0),
                        stop=(k == KH - 1),
                    )
                for k in range(KH):
                    nc.tensor.matmul(
                        pb[:],
                        bsrc[:, bbase + (KH + k) * 128:bbase + (KH + k + 1) * 128],
                        xk(k),
                        start=(k == 0),
                        stop=(k == KH - 1),
                    )
                sa = sap.tile([128, C], F32, tag="sa")
                nc.scalar.activation(
                    sa[:], pa[:], mybir.ActivationFunctionType.Silu
                )
                if j >= BQ0:
                    # b was stored *64 instead of /64: fold 2^-12 into sa.
                    nc.vector.tensor_scalar_mul(sa[:], sa[:], 2.0 ** -12)
                nc.vector.tensor_mul(it_all[:, j * C:(j + 1) * C], sa[:], pb[:])

                # Split-contraction GEMM2, half 1 (k-tiles 0-7, i.e. pairs
                # 0-7 of it_all, all ready once pair 7 retired): one h-block
                # slice after each of pairs 8-15. This fills the PE stalls
                # the bf16/fp8 weight stream cannot avoid (pair demand is
                # ~1.4x the stream rate) and keeps HAM at full clock; the
                # f32 partials park in yh.
                if j >= NPAIR - MH:
                    h = j - (NPAIR - MH)
                    py = ps2.tile([128, C], F32, tag="py")
                    for ki in range(KI // 2):
                        nc.tensor.matmul(
                            py[:],
                            w2t[:, h * HB2 + ki * 128:h * HB2 + (ki + 1) * 128],
                            it_all[:, ki * C:(ki + 1) * C],
                            start=(ki == 0),
                            stop=(ki == KI // 2 - 1),
                        )
                    nc.vector.tensor_copy(yh[:, h * C:(h + 1) * C], py[:])

            # GEMM2 half 2 (k-tiles 8-15): consumes the stream tail at the
            # rate it arrives; DVE adds the half1 partials and casts to
            # bf16. Stores every 2 h-blocks on the scalar ring (baseline
            # store structure) so they never head-block the weight stream.
            for hc in range(0, MH, 2):
                yt = outp.tile([128, 2 * C], BF16, tag="yt")
                for hh in range(2):
                    h = hc + hh
                    py = ps2.tile([128, C], F32, tag="py")
                    for ki in range(KI // 2):
                        nc.tensor.matmul(
                            py[:],
                            w2t[:, H2 + h * HB2 + ki * 128:H2 + h * HB2 + (ki + 1) * 128],
                            it_all[:, (KI // 2 + ki) * C:(KI // 2 + ki + 1) * C],
                            start=(ki == 0),
                            stop=(ki == KI // 2 - 1),
                        )
                    nc.vector.tensor_add(
                        yt[:, hh * C:(hh + 1) * C], yh[:, h * C:(h + 1) * C], py[:]
                    )
                nc.scalar.dma_start(yT_d[:, hc * C:(hc + 2) * C], yt[:])
    nc.compile()
    return nc


def _get_program(C):
    if C not in _compiled:
        _compiled[C] = _build_program(C)
    return _compiled[C]


def _relayout_w1(w1_e):
    # w1_e: [H, 2I] bf16 (b-half of pairs < BQ0 pre-scaled by 1/64) ->
    # [128, BQ0*PAIR_COLS + NFP8B*BQ_COLS]: pairs 0..BQ0-1 hold a_j's 8
    # k-tiles then b_j's; pairs BQ0.. hold only a_j (their b is fp8 in
    # blob2). Stationary [K=128, M=128] layout (partition = contraction row).
    A = w1_e[:, :I].reshape(KH, 128, NPAIR, 128)
    Bh = w1_e[:, I:].reshape(KH, 128, NPAIR, 128)
    pairs = np.stack([A[:, :, :BQ0], Bh[:, :, :BQ0]], axis=0)
    full = pairs.transpose(2, 3, 0, 1, 4).reshape(128, BQ0 * PAIR_COLS)
    atail = A[:, :, BQ0:].transpose(1, 2, 0, 3).reshape(128, NFP8B * BQ_COLS)
    return np.ascontiguousarray(np.concatenate([full, atail], axis=1))


def _relayout_w1bq(bq_e):
    # bq_e: [H, NFP8B*128] fp8 (*64) -> [128, NFP8B*BQ_COLS], per-pair the
    # 8 k-tiles in stationary layout.
    r = bq_e.reshape(KH, 128, NFP8B, 128)
    return np.ascontiguousarray(
        r.transpose(1, 2, 0, 3).reshape(128, NFP8B * BQ_COLS)
    )


def _relayout_w2(w2_e):
    # w2_e: [I, H] fp8 -> [128, MH*HB_COLS] in split-contraction order:
    # [half1 = h-major k-tiles 0-7 | half2 = h-major k-tiles 8-15], each
    # h slice holding its 8 stationary k-tiles in consumption order.
    r = w2_e.reshape(KI, 128, MH, 128)
    h1 = r[:KI // 2].transpose(1, 2, 0, 3).reshape(128, MH * KI // 2 * 128)
    h2 = r[KI // 2:].transpose(1, 2, 0, 3).reshape(128, MH * KI // 2 * 128)
    return np.ascontiguousarray(np.concatenate([h1, h2], axis=1))


def kernel(hidden_states, tokens_per_expert, w1, w2):
    x = np.asarray(hidden_states).reshape(-1, H)
    flat = np.asarray(tokens_per_expert).reshape(-1).astype(np.int64)
    w1 = np.asarray(w1)
    w2 = np.asarray(w2)
    n_rows = flat.shape[0]

    order = np.argsort(flat, kind="stable")
    token_of_row = order // TOPK
    counts = np.bincount(flat, minlength=E)
    starts = np.concatenate([[0], np.cumsum(counts)[:-1]])

    x_bf = x.astype(NP_BF16)
    if w1.dtype != NP_BF16:
        w1 = w1.astype(NP_BF16)

    C = max(48, int(-(-int(counts.max()) // 2)) * 2)
    XCOLS = KH * C
    nc = _get_program(C)

    # b-half of bf16 pairs scaled by 1/W2_SCALE (exponent shift, lossless);
    # w2 and the trailing b-halves stored as e3m4 * W2_SCALE (the latter
    # compensated by 2^-12 on sa in-kernel).
    w1s = np.concatenate(
        [w1[:, :, :I], (w1[:, :, I:].astype(np.float32) / W2_SCALE).astype(NP_BF16)],
        axis=2,
    )
    bq = (
        w1[:, :, I + BQ0 * 128:].astype(np.float32) * W2_SCALE
    ).astype(NP_FP8)
    w2q = (w2.astype(np.float32) * W2_SCALE).astype(NP_FP8)
    w1r = [_relayout_w1(w1s[e]) for e in range(E)]
    bqr = [_relayout_w1bq(bq[e]) for e in range(E)]
    w2r = [_relayout_w2(w2q[e]) for e in range(E)]

    out = np.zeros((n_rows, H), dtype=NP_BF16)
    LAST_RUNS.clear()
    n_waves = int(max(1, -(-int(counts.max()) // C)))
    for wave in range(n_waves):
        in_maps = []
        for e in range(E):
            lo = starts[e] + wave * C
            cnt = int(min(C, max(0, counts[e] - wave * C)))
            xe = np.zeros((C, H), dtype=NP_BF16)
            if cnt:
                xe[:cnt] = x_bf[token_of_row[lo:lo + cnt]]
            # xT layout: [128, KH*C], k-tile k at cols [k*C, (k+1)*C):
            # xT[p, k*C + c] = xe[c, k*128 + p]
            xT = np.ascontiguousarray(
                xe.T.reshape(KH, 128, C).transpose(1, 0, 2).reshape(128, XCOLS)
            )
            blob1 = np.concatenate(
                [
                    xT[:, :XCOLS // 2],        # x k0-3
                    w1r[e][:, :512],           # a0 k0-3
                    xT[:, XCOLS // 2:],        # x k4-7
                    w1r[e][:, 512:1024],       # a0 k4-7
                    w1r[e][:, 1024:2048],      # b0
                    w1r[e][:, 2048:],          # pairs 1.. + a-tail
                ],
                axis=1,
            )
            blob2 = np.concatenate([bqr[e], w2r[e]], axis=1)
            in_maps.append({"blob1": blob1, "blob2": blob2})

        res = run_bass_kernel_spmd(nc, in_maps, list(range(N_CORES)))
        LAST_RUNS.append(res)
        for e in range(E):
            lo = starts[e] + wave * C
            cnt = int(min(C, max(0, counts[e] - wave * C)))
            if not cnt:
                continue
            yT = res.results[e]["yT"]
            # yT[p, h*C + c] = y[c, h*128 + p]
            y = yT.reshape(128, MH, C).transpose(2, 1, 0).reshape(C, H)
            out[lo:lo + cnt] = y[:cnt]
    return out


# revision 23
# speedup vs baseline: 1.1425x; 1.1099x over previous
"""MoE expert MLP (SwiGLU, top-2 routing) on 8 Trainium2 NeuronCores.

Strategy: expert-parallel. Host routes tokens (stable argsort by expert id,
matching the reference), gathers each expert's token rows, and pads them to a
fixed capacity C. Core e runs expert e's two GEMMs + SwiGLU over its C-column
token panel; the host scatters results back into the permuted [N, H] output.

Design (trace-driven; exec_time = last-epilogue-inst-end minus first-main-
block-inst; the ~6us framework preamble is excluded but the fixed ~8.3us
256-semaphore wipe epilogue IS counted, so only the work window matters):
  - All per-core input bytes stream in exact PE consumption order on the
    sync HWDGE ring: [xT+a0 (raw pre-chunk) | x-tail+a0-tail+b0 | w1
    pairs | a-tail + fp8 b-tail interleaved with w2-half1 | w2-half2].
    Per-chunk column-slice DMAs into persistent tiles give Tile-tracked
    gating; chunk count and store structure match the long-proven session
    baseline exactly (26 input DMAs + 4 stores) — a variant with 8
    per-h-block stores alternating sync/scalar rings and 12 w2 chunks hit
    nondeterministic NRT_EXEC_UNIT_UNRECOVERABLE hangs (2 of 9 runs).
  - PE warmup: 9 garbage N=512 matmuls issued before the input-gate wait
    run during the 6.9-10.6us framework-preamble/DMA-ramp window, tripping
    the HAM activity monitor so real matmuls run at 2.4GHz from the start
    (without this the PE runs at K=4/8 = 1.2GHz until ~18us: +~3us). Only
    18 Tensor-stream instructions: queue 14 (engine instruction fetch,
    16KB chunks) rides exclusively on the first SDMA engine of the core's
    block and steals from its input bandwidth mid-stream, so program size
    is kept small.
  - Split-contraction GEMM2: half 1 (it_all k-tiles 0-7, ready after pair
    7) runs as one h-block slice interleaved after each of pairs 8-15,
    parking f32 partials in SBUF; half 2 + DVE add + bf16 cast run after
    the pair loop. During the pair loop the weight stream cannot feed the
    warm PE (pair demand ~467GB/s bf16 / ~364GB/s fp8 vs ~330GB/s
    delivered), so without the interleave the PE idles in 1-3us slices
    (risking HAM re-throttle) and all 8.4us of GEMM2 serializes after the
    stream; with it the stalls are filled and only ~4.2us of half-2 GEMM2
    (consuming the tail at its ~250GB/s demand rate) remains at the end.
  - Quantization (e3m4 = TRN FP8_EXP3, 4 mantissa bits): all of w2 and the
    b-halves of pairs 4-15 (NFP8B=12) are stored *64 in fp8, cutting 3.7MB
    (~28%) off the 9.2MB stream. The 1/64 compensation folds into the bf16
    b-halves (exponent shift, lossless) and a 2^-12 scalar on sa for the
    fp8-b pairs. Measured rel err 0.018656 vs the 0.02 gate (NFP8B=14 ->
    0.0193 leaves <4% margin - declined).
  - C=142 = the exact max expert load for this routing (ceil to mult of 2).
  - GEMM2 stores per h-block, alternating the scalar/sync HWDGE rings so
    the final store's trigger never queues behind earlier store triggers;
    the ~1.3us HBM write-receipt of the last store is the irreducible tail.

Per-core dataflow (all activations transposed, tokens on the free dim):
  GEMM1:  h1T[m-block j] = w1[e][:, cols].T @ xT      (8 H k-tiles, PSUM acc)
  SwiGLU: interT[j] = silu(h1T_a[j]) * h1T_b[j]       (ACT + DVE out of PSUM)
  GEMM2:  yT[h-block]  = w2q[e][:, cols].T @ interT   (16 I k-tiles, fp8 lhsT)
"""

import numpy as np
import ml_dtypes

import concourse.bass as bass
import concourse.mybir as mybir
import concourse.tile as tile
from concourse import bacc
from concourse.bass_utils import run_bass_kernel_spmd

BF16 = mybir.dt.bfloat16
FP8 = mybir.dt.float8e3
F32 = mybir.dt.float32
NP_BF16 = ml_dtypes.bfloat16
NP_FP8 = ml_dtypes.float8_e3m4

# Problem shape (hardcoded per the contract; matches nn_Experts_41429254537622)
B, S, H, I, E, TOPK = 1, 512, 1024, 2048, 8, 2
N_CORES = 8
KH = H // 128    # 8  k-tiles for GEMM1 (contraction over H)
NPAIR = I // 128 # 16 (a, b) pairs of 128-wide w1 column blocks
KI = I // 128    # 16 k-tiles for GEMM2 (contraction over I)
MH = H // 128    # 8  output row blocks of yT
PAIR_COLS = 2 * KH * 128   # 2048 w1 blob columns per (a, b) pair block
HB_COLS = KI * 128         # 2048 w2 blob columns per h block (fp8: 1B each)
W2_SCALE = 64.0            # w2 stored as e3m4 * 64; 1/64 folded into w1's b half
NFP8B = 12                 # trailing pairs whose b half is stored fp8 (b*64)
BQ0 = NPAIR - NFP8B        # first fp8-b pair
BQ_COLS = KH * 128         # 1024 fp8 b columns per pair
NWARM = 9                  # PE warmup matmuls, N=512 (~3.8us at cold 1.2GHz)
# Pair COMPUTE order: pair 0 (head), then the fp8-b pairs (0.39MB/pair ->
# ~368GB/s PE demand) during the slow early stream ramp, then the bf16
# pairs 1-3 (0.5MB/pair -> ~467GB/s) late, where interleaved GEMM2 slices
# fill their inevitable stalls. GEMM2 half1 contracts over the first 8
# computed pairs, half2 over the last 8.
PERM = [0] + list(range(BQ0, NPAIR)) + [1, 2, 3]
HALF1 = PERM[:NPAIR // 2]  # it_all k-tiles of GEMM2 half 1
HALF2 = PERM[NPAIR // 2:]  # it_all k-tiles of GEMM2 half 2

_compiled = {}
LAST_RUNS = []  # BassKernelResults of the most recent kernel() call (for test harness)


def _build_program(C):
    XCOLS = KH * C
    nc = bacc.Bacc(
        "TRN2", target_bir_lowering=False, debug=False, num_devices=N_CORES
    )
    # blob1: [ xT (XCOLS) | pair0 a+b (PAIR_COLS) | pairs 1..15 ] in exact
    # consumption order. blob2: w2 as fp8, h-block-major.
    blob1 = nc.dram_tensor(
        "blob1",
        [128, XCOLS + BQ0 * PAIR_COLS + NFP8B * BQ_COLS],
        BF16,
        kind="ExternalInput",
    )
    blob2 = nc.dram_tensor(
        "blob2", [128, NFP8B * BQ_COLS + MH * HB_COLS], FP8,
        kind="ExternalInput",
    )
    yT_d = nc.dram_tensor("yT", [128, MH * C], BF16, kind="ExternalOutput")

    # Pre-Tile raw load: the PE's first work (pair 0's a-chain) needs only
    # x k0-3 + a0 k0-3, so that prefix is its own DMA streamed during the
    # framework preamble. The rest of the head rides as the first Tile
    # chunks (Tile gates the a-chain mid-accumulation, which is legal).
    # blob1 head: [x k0-3 (4C) | a0 k0-3 (512) | x k4-7 (4C) | a0 k4-7
    # (512) | b0 (1024)]
    HC = XCOLS // 2                 # 4C
    P0A = HC + 4 * 128              # end of [x k0-3 | a0 k0-3]
    P0B = XCOLS + PAIR_COLS         # end of the whole head
    xw0_raw = nc.alloc_sbuf_tensor("xw0_pre", [128, P0A], BF16)
    warm_sb = nc.alloc_sbuf_tensor("warm_sb", [128, 512 + 128], BF16)
    pre_sem = nc.alloc_semaphore(name="pre_dma_sem")
    xw0 = xw0_raw.ap()
    nc.sync.dma_start(xw0[:, :P0A], blob1[:, :P0A]).then_inc(pre_sem, 16)

    # PE warmup: wide (N=512) garbage matmuls into an aliased PSUM bank.
    # They run during the preamble + head-DMA window (strictly before every
    # real matmul in PE program order) and hold the PE busy >3.4us so HAM
    # unthrottles the clock to 2.4GHz before real work starts. Bank 7 is
    # reused by a Tile PSUM pool much later; PE-queue ordering makes the
    # aliasing safe. N=512 keeps this at 9 LDWEIGHTS+MATMUL pairs so the
    # Tensor instruction stream (fetched over the straggler-prone first
    # SDMA engine) stays small.
    warm_ps = nc.place_psum_tensor("warm_ps", [128, 512], F32, bank=7)
    wps = warm_ps.ap()
    wsb = warm_sb.ap()
    for _ in range(NWARM):
        nc.tensor.matmul(
            wps[:, :], wsb[:, 512:640], wsb[:, :512], start=True, stop=True
        )
    nc.tensor.wait_ge(pre_sem, 16)

    with tile.TileContext(nc) as tc:
        with (
            tc.tile_pool(name="wp", bufs=1) as wp,
            tc.tile_pool(name="sap", bufs=4) as sap,
            tc.tile_pool(name="outp", bufs=2) as outp,
            tc.tile_pool(name="ps1", bufs=6, space="PSUM") as ps1,
            tc.tile_pool(name="ps2", bufs=2, space="PSUM") as ps2,
        ):
            # Persistent weight panels + inter panel: DMAs write disjoint
            # column slices, matmuls read sub-slices; Tile's range tracker
            # gives per-chunk gating with no buffer rotation or WAR stalls.
            # w1t holds [pairs 1..3 full | a4..a15]; b4..b15 live fp8 in
            # w1bq (stored *64, compensated by 2^-12 on sa).
            # xb1 carries [x k4-7 | a0 k4-7 | b0] as the first Tile chunks.
            # w2t is split-contraction: [half1 = per-h k-tiles 0-7 | half2 =
            # per-h k-tiles 8-15]; yh holds the f32 half1 partial sums.
            xb1 = wp.tile([128, HC + 4 * 128 + KH * 128], BF16)
            w1t = wp.tile(
                [128, (BQ0 - 1) * PAIR_COLS + NFP8B * BQ_COLS], BF16
            )
            w1bq = wp.tile([128, NFP8B * BQ_COLS], FP8)
            w2t = wp.tile([128, MH * HB_COLS], FP8)
            it_all = wp.tile([128, KI * C], BF16)
            yh = wp.tile([128, MH * C], F32)
            A8 = (BQ0 - 1) * PAIR_COLS          # w1t col of the a-tail
            HB2 = KI // 2 * 128                 # 1024 w2 cols per (h, half)
            H2 = MH * HB2                       # w2t col base of half2

            def xk(k):
                # x k-tile k: first half in the pre tensor, rest in xb1.
                if k < 4:
                    return xw0[:, k * C:(k + 1) * C]
                return xb1[:, (k - 4) * C:(k - 3) * C]

            # Head tail as two Tile chunks (split so pair 0's k4 gate lands
            # before b0), then the stream follows the PERM compute order:
            # fp8 a/b chunks (with w2-half1 2h-chunks interleaved once the
            # matching GEMM2 slices become eligible), then the bf16 pairs
            # 1..3 with the last w2-half1 chunk, then w2 half2.
            XB_SPLIT = HC + 4 * 128
            nc.sync.dma_start(xb1[:, :XB_SPLIT], blob1[:, P0A:P0A + XB_SPLIT])
            nc.sync.dma_start(
                xb1[:, XB_SPLIT:], blob1[:, P0A + XB_SPLIT:P0B]
            )
            W1BASE = P0B                        # blob1 col of pair 1
            AB = W1BASE + (BQ0 - 1) * PAIR_COLS  # blob1 col of the a-tail
            W2B = NFP8B * BQ_COLS               # blob2 col of w2
            for jj in range(0, NFP8B, 2):
                nc.sync.dma_start(
                    w1t[:, A8 + jj * BQ_COLS:A8 + (jj + 2) * BQ_COLS],
                    blob1[:, AB + jj * BQ_COLS:AB + (jj + 2) * BQ_COLS],
                )
                nc.sync.dma_start(
                    w1bq[:, jj * BQ_COLS:(jj + 2) * BQ_COLS],
                    blob2[:, jj * BQ_COLS:(jj + 2) * BQ_COLS],
                )
                if jj >= 6:
                    # w2 half1 for h-blocks (jj-6, jj-5): the h0 slice runs
                    # after computed pair 11 (in the jj=6 chunk), h2 after
                    # pair 13 (jj=8), h4 after pair 15 (jj=10).
                    hb = (jj - 6) * HB2
                    nc.sync.dma_start(
                        w2t[:, hb:hb + 2 * HB2],
                        blob2[:, W2B + hb:W2B + hb + 2 * HB2],
                    )
            nc.sync.dma_start(
                w1t[:, 0:PAIR_COLS], blob1[:, W1BASE:W1BASE + PAIR_COLS]
            )
            nc.sync.dma_start(
                w2t[:, 6 * HB2:8 * HB2],
                blob2[:, W2B + 6 * HB2:W2B + 8 * HB2],
            )
            for p0 in range(2, BQ0):
                nc.sync.dma_start(
                    w1t[:, (p0 - 1) * PAIR_COLS:p0 * PAIR_COLS],
                    blob1[:, W1BASE + (p0 - 1) * PAIR_COLS:W1BASE + p0 * PAIR_COLS],
                )
            for h in range(0, MH, 2):
                nc.sync.dma_start(
                    w2t[:, H2 + h * HB2:H2 + (h + 2) * HB2],
                    blob2[:, W2B + H2 + h * HB2:W2B + H2 + (h + 2) * HB2],
                )

            # GEMM1 + SwiGLU, pair-by-pair in PERM (= stream) order.
            for i, j in enumerate(PERM):
                if j == 0:
                    asrc, abase = None, 0            # per-k split below
                    bsrc, bbase = xb1, HC - 4 * 128  # b0 at xb1[HC+512:]
                elif j < BQ0:
                    asrc, abase = w1t, (j - 1) * PAIR_COLS
                    bsrc, bbase = w1t, (j - 1) * PAIR_COLS
                else:
                    asrc, abase = w1t, A8 + (j - BQ0) * BQ_COLS
                    bsrc, bbase = w1bq, (j - BQ0) * BQ_COLS - KH * 128
                pa = ps1.tile([128, C], F32, tag="pab")
                pb = ps1.tile([128, C], F32, tag="pab")
                for k in range(KH):
                    if j == 0:
                        if k < 4:
                            ak = xw0[:, HC + k * 128:HC + (k + 1) * 128]
                        else:
                            ak = xb1[:, HC + (k - 4) * 128:HC + (k - 3) * 128]
                    else:
                        ak = asrc[:, abase + k * 128:abase + (k + 1) * 128]
                    nc.tensor.matmul(
                        pa[:],
                        ak,
                        xk(k),
                        start=(k == 0),
                        stop=(k == KH - 1),
                    )
                for k in range(KH):
                    nc.tensor.matmul(
                        pb[:],
                        bsrc[:, bbase + (KH + k) * 128:bbase + (KH + k + 1) * 128],
                        xk(k),
                        start=(k == 0),
                        stop=(k == KH - 1),
                    )
                sa = sap.tile([128, C], F32, tag="sa")
                nc.scalar.activation(
                    sa[:], pa[:], mybir.ActivationFunctionType.Silu
                )
                if j >= BQ0:
                    # b was stored *64 instead of /64: fold 2^-12 into sa.
                    nc.vector.tensor_scalar_mul(sa[:], sa[:], 2.0 ** -12)
                nc.vector.tensor_mul(it_all[:, j * C:(j + 1) * C], sa[:], pb[:])

                # Split-contraction GEMM2, half 1 (the HALF1 k-tiles of
                # it_all, all ready once the 8th computed pair retired):
                # one h-block slice after each of the last 8 computed
                # pairs. This fills the PE stalls the weight stream cannot
                # avoid (pair demand exceeds the stream rate) and keeps HAM
                # at full clock; the f32 partials park in yh.
                if i >= NPAIR - MH:
                    h = i - (NPAIR - MH)
                    py = ps2.tile([128, C], F32, tag="py")
                    for ki, jk in enumerate(HALF1):
                        nc.tensor.matmul(
                            py[:],
                            w2t[:, h * HB2 + ki * 128:h * HB2 + (ki + 1) * 128],
                            it_all[:, jk * C:(jk + 1) * C],
                            start=(ki == 0),
                            stop=(ki == KI // 2 - 1),
                        )
                    nc.vector.tensor_copy(yh[:, h * C:(h + 1) * C], py[:])

            # GEMM2 half 2 (k-tiles 8-15): consumes the stream tail at the
            # rate it arrives; DVE adds the half1 partials and casts to
            # bf16. Stores every 2 h-blocks on the scalar ring (baseline
            # store structure) so they never head-block the weight stream.
            for hc in range(0, MH, 2):
                yt = outp.tile([128, 2 * C], BF16, tag="yt")
                for hh in range(2):
                    h = hc + hh
                    py = ps2.tile([128, C], F32, tag="py")
                    for ki, jk in enumerate(HALF2):
                        nc.tensor.matmul(
                            py[:],
                            w2t[:, H2 + h * HB2 + ki * 128:H2 + h * HB2 + (ki + 1) * 128],
                            it_all[:, jk * C:(jk + 1) * C],
                            start=(ki == 0),
                            stop=(ki == KI // 2 - 1),
                        )
                    nc.vector.tensor_add(
                        yt[:, hh * C:(hh + 1) * C], yh[:, h * C:(h + 1) * C], py[:]
                    )
                nc.scalar.dma_start(yT_d[:, hc * C:(hc + 2) * C], yt[:])
    nc.compile()
    return nc


def _get_program(C):
    if C not in _compiled:
        _compiled[C] = _build_program(C)
    return _compiled[C]


def _relayout_w1(w1_e):
    # w1_e: [H, 2I] bf16 (b-half of pairs < BQ0 pre-scaled by 1/64) ->
    # [128, BQ0*PAIR_COLS + NFP8B*BQ_COLS]: pairs 0..BQ0-1 hold a_j's 8
    # k-tiles then b_j's; pairs BQ0.. hold only a_j (their b is fp8 in
    # blob2). Stationary [K=128, M=128] layout (partition = contraction row).
    A = w1_e[:, :I].reshape(KH, 128, NPAIR, 128)
    Bh = w1_e[:, I:].reshape(KH, 128, NPAIR, 128)
    pairs = np.stack([A[:, :, :BQ0], Bh[:, :, :BQ0]], axis=0)
    full = pairs.transpose(2, 3, 0, 1, 4).reshape(128, BQ0 * PAIR_COLS)
    atail = A[:, :, BQ0:].transpose(1, 2, 0, 3).reshape(128, NFP8B * BQ_COLS)
    return np.ascontiguousarray(np.concatenate([full, atail], axis=1))


def _relayout_w1bq(bq_e):
    # bq_e: [H, NFP8B*128] fp8 (*64) -> [128, NFP8B*BQ_COLS], per-pair the
    # 8 k-tiles in stationary layout.
    r = bq_e.reshape(KH, 128, NFP8B, 128)
    return np.ascontiguousarray(
        r.transpose(1, 2, 0, 3).reshape(128, NFP8B * BQ_COLS)
    )


def _relayout_w2(w2_e):
    # w2_e: [I, H] fp8 -> [128, MH*HB_COLS] in split-contraction order:
    # [half1 = h-major k-tiles HALF1 | half2 = h-major k-tiles HALF2]
    # (k-tile j of GEMM2 = w2 row block j = intermediate pair j), each h
    # slice holding its 8 stationary k-tiles in consumption order.
    r = w2_e.reshape(KI, 128, MH, 128)
    h1 = r[HALF1].transpose(1, 2, 0, 3).reshape(128, MH * KI // 2 * 128)
    h2 = r[HALF2].transpose(1, 2, 0, 3).reshape(128, MH * KI // 2 * 128)
    return np.ascontiguousarray(np.concatenate([h1, h2], axis=1))


def kernel(hidden_states, tokens_per_expert, w1, w2):
    x = np.asarray(hidden_states).reshape(-1, H)
    flat = np.asarray(tokens_per_expert).reshape(-1).astype(np.int64)
    w1 = np.asarray(w1)
    w2 = np.asarray(w2)
    n_rows = flat.shape[0]

    order = np.argsort(flat, kind="stable")
    token_of_row = order // TOPK
    counts = np.bincount(flat, minlength=E)
    starts = np.concatenate([[0], np.cumsum(counts)[:-1]])

    x_bf = x.astype(NP_BF16)
    if w1.dtype != NP_BF16:
        w1 = w1.astype(NP_BF16)

    C = max(48, int(-(-int(counts.max()) // 2)) * 2)
    XCOLS = KH * C
    nc = _get_program(C)

    # b-half of bf16 pairs scaled by 1/W2_SCALE (exponent shift, lossless);
    # w2 and the trailing b-halves stored as e3m4 * W2_SCALE (the latter
    # compensated by 2^-12 on sa in-kernel).
    w1s = np.concatenate(
        [w1[:, :, :I], (w1[:, :, I:].astype(np.float32) / W2_SCALE).astype(NP_BF16)],
        axis=2,
    )
    bq = (
        w1[:, :, I + BQ0 * 128:].astype(np.float32) * W2_SCALE
    ).astype(NP_FP8)
    w2q = (w2.astype(np.float32) * W2_SCALE).astype(NP_FP8)
    w1r = [_relayout_w1(w1s[e]) for e in range(E)]
    bqr = [_relayout_w1bq(bq[e]) for e in range(E)]
    w2r = [_relayout_w2(w2q[e]) for e in range(E)]

    out = np.zeros((n_rows, H), dtype=NP_BF16)
    LAST_RUNS.clear()
    n_waves = int(max(1, -(-int(counts.max()) // C)))
    for wave in range(n_waves):
        in_maps = []
        for e in range(E):
            lo = starts[e] + wave * C
            cnt = int(min(C, max(0, counts[e] - wave * C)))
            xe = np.zeros((C, H), dtype=NP_BF16)
            if cnt:
                xe[:cnt] = x_bf[token_of_row[lo:lo + cnt]]
            # xT layout: [128, KH*C], k-tile k at cols [k*C, (k+1)*C):
            # xT[p, k*C + c] = xe[c, k*128 + p]
            xT = np.ascontiguousarray(
                xe.T.reshape(KH, 128, C).transpose(1, 0, 2).reshape(128, XCOLS)
            )
            blob1 = np.concatenate(
                [
                    xT[:, :XCOLS // 2],        # x k0-3
                    w1r[e][:, :512],           # a0 k0-3
                    xT[:, XCOLS // 2:],        # x k4-7
                    w1r[e][:, 512:1024],       # a0 k4-7
                    w1r[e][:, 1024:2048],      # b0
                    w1r[e][:, 2048:],          # pairs 1.. + a-tail
                ],
                axis=1,
            )
            blob2 = np.concatenate([bqr[e], w2r[e]], axis=1)
            in_maps.append({"blob1": blob1, "blob2": blob2})

        res = run_bass_kernel_spmd(nc, in_maps, list(range(N_CORES)))
        LAST_RUNS.append(res)
        for e in range(E):
            lo = starts[e] + wave * C
            cnt = int(min(C, max(0, counts[e] - wave * C)))
            if not cnt:
                continue
            yT = res.results[e]["yT"]
            # yT[p, h*C + c] = y[c, h*128 + p]
            y = yT.reshape(128, MH, C).transpose(2, 1, 0).reshape(C, H)
            out[lo:lo + cnt] = y[:cnt]
    return out
